# revision 1
# baseline (speedup 1.0000x reference)
"""Trainium2 Bass kernel for nn_ContextPromptGenerator.

Math restructure: pooled bins are masked segment sums over tokens, so the
0/1-mask matmul runs FIRST on [T, 4096] packed rows, then the 4096->1024
down-projection runs on [64, 4096] per core.  All operands are plain fp16
(fp32 PSUM): the error gate is 2e-2 and fp16 lands ~4e-4.

Tensor-parallel context path: the embedding rows of ALL samples are
flat-packed evenly across the 8 cores (phase A computes per-core PARTIAL
ctx sums for all 16 samples), an AllReduce combines them, and each core
projects the full ctx sums through its 128-column shard of Wc; an
AllGather reassembles ctx_d [16, 1024].  This removes the 8x-replicated
8.4MB Wc stream (each core now moves 1MB) and the ctx-pack imbalance from
the DMA-bound window.  The collective chain runs on the gpsimd DMA queue
so it never blocks the sync-queue x/weight stream, and phase B's matmuls
are emitted late in X so collective latency hides.

Schedule: A (ctx partials) -> X (bin sums, x stationary / mask moving)
with B (ctxsum @ Wc-shard) interleaved late -> D (@resident Wd + aug,
silu) -> E (transpose) -> U (@Wu + bu, halves double-buffered).

DMA: weights/masks host-reordered partition-major (8KB packets sustain
~430GB/s vs ~170GB/s at 2KB rows).

Sharding: data-parallel over samples (2 per core, paired to minimize
roundup128(max seq pair)); ctx path tensor-parallel as above.
"""

import numpy as np
from contextlib import ExitStack

import concourse.bass as bass
import concourse.mybir as mybir
import concourse.tile as tile
from concourse import bacc
from concourse.masks import make_identity
from concourse.bass_utils import run_bass_kernel_spmd

F32 = mybir.dt.float32
F16 = mybir.dt.float16

B, S, C, H, D, V, P = 16, 2048, 512, 4096, 1024, 32000, 32
NC = 8          # cores
SPC = 2         # samples per core
M = SPC * P     # 64 output rows per core
HT = H // 128   # 32 h-tiles
DT = D // 128   # 8 d-tiles
RG = [[0, 1, 2, 3, 4, 5, 6, 7]]

_cache = {}


def _build(T, Tc):
    """Build the per-core SPMD Bass program.

    T  = packed hidden rows per core (multiple of 128)
    Tc = packed context-embedding rows per core (multiple of 128)
    """
    nc = bacc.Bacc(None, target_bir_lowering=False, num_devices=NC)

    KT = T // 128    # x k-tiles
    KC = Tc // 128   # emb k-tiles

    xh_d = nc.dram_tensor("xh", [T, H], F16, kind="ExternalInput")
    mx_d = nc.dram_tensor("mxr", [128, KT * M], F16, kind="ExternalInput")
    eh_d = nc.dram_tensor("eh", [Tc, H], F16, kind="ExternalInput")
    cm_d = nc.dram_tensor("cmr", [128, KC * B], F16, kind="ExternalInput")
    wcc_d = nc.dram_tensor("wcc", [128, HT * 128], F16, kind="ExternalInput")
    wdh_d = nc.dram_tensor("wdr", [128, HT * D], F16, kind="ExternalInput")
    wuh_d = nc.dram_tensor("wur", [128, 2 * DT * H // 2], F16,
                           kind="ExternalInput")
    bd_d = nc.dram_tensor("bdr", [1, D], F32, kind="ExternalInput")
    bc_d = nc.dram_tensor("bcr", [1, D], F32, kind="ExternalInput")
    bu_d = nc.dram_tensor("bur", [1, H], F16, kind="ExternalInput")
    aug_d = nc.dram_tensor("aug", [B + 2, M], F32, kind="ExternalInput")
    sinv_d = nc.dram_tensor("sinv", [M, 1], F32, kind="ExternalInput")
    out_d = nc.dram_tensor("out", [M, H], F32, kind="ExternalOutput")
    # collective bounce buffers (internal DRAM)
    csp_d = nc.dram_tensor("csp", [128, HT * B], F32)   # partial ctx sums
    csr_d = nc.dram_tensor("csr", [128, HT * B], F32)   # reduced ctx sums
    cdp_d = nc.dram_tensor("cdp", [B, 128], F32)        # ctx_d column shard
    cdg_d = nc.dram_tensor("cdg", [NC * B, 128], F32)   # gathered ctx_d

    WCC = 8 * D       # Wd chunk cols (8 k-tiles per chunk)
    NCHUNK = HT // 8  # 4 chunks

    with tile.TileContext(nc) as tc, ExitStack() as ctx:
        const = ctx.enter_context(tc.tile_pool(name="const", bufs=1))
        big = ctx.enter_context(tc.tile_pool(name="big", bufs=2))
        wupool = ctx.enter_context(tc.tile_pool(name="wupool", bufs=2))
        opool = ctx.enter_context(tc.tile_pool(name="opool", bufs=1))
        keep = ctx.enter_context(tc.tile_pool(name="keep", bufs=1))

        ident16 = const.tile([128, 128], F16)
        idtmp = const.tile([128, 128], F32)
        make_identity(nc, idtmp)
        nc.vector.tensor_copy(ident16, idtmp)
        ones1 = const.tile([1, M], F16)
        nc.vector.memset(ones1, 1.0)
        aug_sb = keep.tile([B + 2, M], F32)
        nc.sync.dma_start(out=aug_sb, in_=aug_d[:, :])
        sinv_sb = keep.tile([M, 1], F32)
        nc.sync.dma_start(out=sinv_sb, in_=sinv_d[:, :])
        # augmented-rhs rows: 0..15 = ctxWcSum[s], 16 = bd, 17 = bc
        augr_sb = keep.tile([B + 2, D], F32)
        nc.sync.dma_start(out=augr_sb[B:B + 1, :], in_=bd_d[:, :])
        nc.sync.dma_start(out=augr_sb[B + 1:B + 2, :], in_=bc_d[:, :])
        mxr_sb = keep.tile([128, KT * M], F16)
        nc.sync.dma_start(out=mxr_sb, in_=mx_d[:, :])
        cmr_sb = keep.tile([128, KC * B], F16)
        nc.sync.dma_start(out=cmr_sb, in_=cm_d[:, :])
        wcc_sb = keep.tile([128, HT * 128], F16)  # Wc column shard, 8KB/part
        nc.sync.dma_start(out=wcc_sb, in_=wcc_d[:, :])
        wd_sb = keep.tile([128, HT * D], F16)     # resident Wd, 64KB/part

        # ---- phase A: partial ctx_sumT[h, s] over this core's emb rows ----
        cs16 = keep.tile([128, HT * B], F16)
        cs32 = keep.tile([128, HT * B], F32)
        with tc.tile_pool(name="psA", bufs=1, space="PSUM") as psA:
            ps_ctx = psA.tile([128, HT * B], F32)  # 2KB/part, 1 bank
            for k in range(KC):
                eht = big.tile([128, H], F16, tag="hih")
                nc.sync.dma_start(out=eht, in_=eh_d[128 * k:128 * (k + 1), :])
                for hc in range(HT):
                    nc.tensor.matmul(
                        ps_ctx[:, B * hc:B * (hc + 1)],
                        eht[:, 128 * hc:128 * (hc + 1)],
                        cmr_sb[:, B * k:B * (k + 1)],
                        start=(k == 0 and hc == 0),
                        stop=(k == KC - 1),
                    )
            nc.vector.tensor_copy(cs32, ps_ctx)
        # AllReduce the partial ctx sums (gpsimd queue; sync queue stays free)
        nc.gpsimd.dma_start(out=csp_d[:, :], in_=cs32)
        nc.gpsimd.collective_compute(
            "AllReduce", mybir.AluOpType.add, replica_groups=RG,
            ins=[csp_d[:, :].opt()], outs=[csr_d[:, :].opt()])
        nc.gpsimd.dma_start(out=cs32, in_=csr_d[:, :])
        nc.vector.tensor_copy(cs16, cs32)

        # ---- phase X: xsumT[h, j] = sum_t x[t, h] * mx01[t, j] ----
        # x tiles stationary, 0/1 mask moving; out 32 slices [128,64].
        # Phase B (ctx_d shard [16,128] = ctxsum @ Wc-cols) is emitted late
        # in X so the AllReduce latency hides under the x stream.
        state = {"mm": 0}

        def emit_b(upto):
            while state["mm"] < min(HT, upto):
                k = state["mm"]
                nc.tensor.matmul(
                    ps_cd,
                    cs16[:, B * k:B * (k + 1)],
                    wcc_sb[:, 128 * k:128 * (k + 1)],
                    start=(k == 0),
                    stop=(k == HT - 1),
                )
                state["mm"] = k + 1

        xs_hi = keep.tile([128, HT * M], F16)
        with tc.tile_pool(name="psX", bufs=1, space="PSUM") as psX, \
                tc.tile_pool(name="psB", bufs=1, space="PSUM") as psB:
            ps_xs = psX.tile([128, HT * M], F32)  # 4 banks, 8 slices per bank
            ps_cd = psB.tile([B, 128], F32)       # 1 bank
            for k in range(KT):
                xht = big.tile([128, H], F16, tag="hih")
                nc.sync.dma_start(out=xht, in_=xh_d[128 * k:128 * (k + 1), :])
                for hc in range(HT):
                    nc.tensor.matmul(
                        ps_xs[:, M * hc:M * (hc + 1)],
                        xht[:, 128 * hc:128 * (hc + 1)],
                        mxr_sb[:, M * k:M * (k + 1)],
                        start=(k == 0 and hc % 8 == 0),
                        stop=(k == KT - 1),
                    )
                if k >= 6:
                    emit_b(3 * (k - 5))
            # resident Wd lands while X's tensor stream drains
            for c in range(NCHUNK):
                nc.sync.dma_start(
                    out=wd_sb[:, WCC * c:WCC * (c + 1)],
                    in_=wdh_d[:, WCC * c:WCC * (c + 1)])
            emit_b(HT)
            for q in range(4):
                nc.vector.tensor_copy(
                    xs_hi[:, 512 * q:512 * (q + 1)],
                    ps_xs[:, 512 * q:512 * (q + 1)])
            # ship this core's ctx_d columns, gather all (gpsimd queue)
            cdp_sb = keep.tile([B, 128], F32)
            nc.vector.tensor_copy(cdp_sb, ps_cd)
            nc.gpsimd.dma_start(out=cdp_d[:, :], in_=cdp_sb)
            nc.gpsimd.collective_compute(
                "AllGather", mybir.AluOpType.bypass, replica_groups=RG,
                ins=[cdp_d[:, :].opt()], outs=[cdg_d[:, :].opt()])
            for i in range(NC):
                nc.gpsimd.dma_start(
                    out=augr_sb[0:B, 128 * i:128 * (i + 1)],
                    in_=cdg_d[B * i:B * (i + 1), :])

        # ---- phase D: pooled[j, d] = xsum.T @ Wd + aug ----
        silu_sb = keep.tile([M, D], F16)
        with tc.tile_pool(name="psD", bufs=1, space="PSUM") as psD:
            ps_pool = psD.tile([M, D], F32)  # 2 banks
            for k in range(HT):
                for nb in range(2):
                    nc.tensor.matmul(
                        ps_pool[:, 512 * nb:512 * (nb + 1)],
                        xs_hi[:, M * k:M * (k + 1)],
                        wd_sb[:, D * k + 512 * nb:D * k + 512 * (nb + 1)],
                        start=(k == 0),
                        stop=False,
                    )
            for nb in range(2):
                nc.tensor.matmul(
                    ps_pool[:, 512 * nb:512 * (nb + 1)],
                    aug_sb,
                    augr_sb[:, 512 * nb:512 * (nb + 1)],
                    start=False, stop=True,
                )
            # scale by 1/S and silu in one ACT op per bank
            for nb in range(2):
                nc.scalar.activation(
                    silu_sb[:, 512 * nb:512 * (nb + 1)],
                    ps_pool[:, 512 * nb:512 * (nb + 1)],
                    mybir.ActivationFunctionType.Silu,
                    scale=sinv_sb,
                )

        # ---- phase E: siluT slices [128, 64] per d-tile ----
        sT_hi = keep.tile([128, DT * M], F16)
        with tc.tile_pool(name="psE", bufs=2, space="PSUM") as psE:
            for dc in range(DT):
                pst = psE.tile([128, M], F16, tag="silutr")
                nc.tensor.transpose(
                    pst, silu_sb[:, 128 * dc:128 * (dc + 1)],
                    ident16[0:M, 0:M])
                nc.vector.tensor_copy(sT_hi[:, M * dc:M * (dc + 1)], pst)

        # ---- phase U: out[j, h] = siluT.T @ Wu + bu ----
        HH = H // 2  # two halves to keep psum at 4 banks
        with tc.tile_pool(name="psU", bufs=2, space="PSUM") as psU:
            for half in range(2):
                wut = wupool.tile([128, DT * HH], F16, tag="wuh")  # 32KB/part
                for c in range(2):
                    nc.sync.dma_start(
                        out=wut[:, DT * HH // 2 * c:DT * HH // 2 * (c + 1)],
                        in_=wuh_d[:, DT * HH * half + DT * HH // 2 * c:
                                  DT * HH * half + DT * HH // 2 * (c + 1)])
                but = opool.tile([1, HH], F16, tag="bu")
                nc.sync.dma_start(
                    out=but, in_=bu_d[:, HH * half:HH * (half + 1)])
                ps_out = psU.tile([M, HH], F32, tag="outps")  # 4 banks
                for dc in range(DT):
                    for nb in range(HH // 512):
                        nc.tensor.matmul(
                            ps_out[:, 512 * nb:512 * (nb + 1)],
                            sT_hi[:, M * dc:M * (dc + 1)],
                            wut[:, HH * dc + 512 * nb:HH * dc + 512 * (nb + 1)],
                            start=(dc == 0),
                            stop=False,
                        )
                for nb in range(HH // 512):
                    nc.tensor.matmul(
                        ps_out[:, 512 * nb:512 * (nb + 1)],
                        ones1,
                        but[:, 512 * nb:512 * (nb + 1)],
                        start=False, stop=True,
                    )
                ot = opool.tile([M, HH], F32, tag="ot")
                for nb in range(HH // 512):
                    nc.vector.tensor_copy(
                        ot[:, 512 * nb:512 * (nb + 1)],
                        ps_out[:, 512 * nb:512 * (nb + 1)])
                nc.sync.dma_start(
                    out=out_d[:, HH * half:HH * (half + 1)], in_=ot)

    nc.finalize()
    return nc


def _roundup(v, m):
    return max(m, ((int(v) + m - 1) // m) * m)


def _pm(a, kt):
    """Reorder [kt*128, cols] row-major -> partition-major [128, kt*cols]."""
    n, cols = a.shape
    assert n == kt * 128
    return np.ascontiguousarray(
        a.reshape(kt, 128, cols).transpose(1, 0, 2).reshape(128, kt * cols))


def _pair_samples(seq):
    """Pair the 16 samples 2-per-core minimizing roundup128(max pair seq).
    Greedy sort-and-reflect, then 2-opt passes."""
    order = np.argsort(-seq, kind="stable")
    pairs = [[int(order[i]), int(order[2 * NC - 1 - i])] for i in range(NC)]

    def cost(ps):
        return (_roundup(max(seq[a] + seq[b] for a, b in ps), 128),
                max(seq[a] + seq[b] for a, b in ps))

    best = cost(pairs)
    improved = True
    while improved:
        improved = False
        for i in range(NC):
            for j in range(i + 1, NC):
                for swap in ((1, 1), (1, 0), (0, 1)):
                    cand = [list(p) for p in pairs]
                    cand[i][swap[0]], cand[j][swap[1]] = \
                        cand[j][swap[1]], cand[i][swap[0]]
                    c = cost(cand)
                    if c < best:
                        best, pairs, improved = c, cand, True
    return [(a, b) for a, b in pairs]


def kernel(**inputs):
    ids = np.asarray(inputs["context_ids"]).astype(np.int64)
    x = np.asarray(inputs["hidden_states"], dtype=np.float32)
    seq = np.asarray(inputs["seq_lengths"]).astype(np.int64)
    clen = np.asarray(inputs["context_lengths"]).astype(np.int64)
    emb = np.asarray(inputs["embed_table"], dtype=np.float32)
    Wc = np.ascontiguousarray(inputs["Wc"], dtype=np.float32)
    bc = np.asarray(inputs["bc"], dtype=np.float32)
    Wd = np.ascontiguousarray(inputs["Wd"], dtype=np.float32)
    bd = np.asarray(inputs["bd"], dtype=np.float32)
    Wu = np.ascontiguousarray(inputs["Wu"], dtype=np.float32)
    bu = np.asarray(inputs["bu"], dtype=np.float32)

    assert x.shape == (B, S, H) and ids.shape == (B, C)

    # per-sample bin geometry
    L = seq + 1
    jj = np.arange(P, dtype=np.int64)
    start = (jj[None, :] * L[:, None]) // P            # [B,P]
    end = ((jj[None, :] + 1) * L[:, None] + P - 1) // P
    Sj = (end - start).astype(np.float32)
    lo = np.maximum(start - 1, 0)
    hi = end - 1
    cnt = (hi - lo).astype(np.float32)
    ind = (start == 0).astype(np.float32)

    pairs = _pair_samples(seq)
    T = _roundup(max(seq[a] + seq[b] for a, b in pairs), 128)
    KT = T // 128

    # flat-pack ALL samples' context-embedding rows evenly across cores
    clen_c = np.maximum(clen, 1)
    labels = np.concatenate([np.full(int(clen_c[s]), s, np.int64)
                             for s in range(B)])
    rows = np.concatenate([ids[s, :int(clen_c[s])] for s in range(B)])
    tot = len(labels)
    chunk = -(-tot // NC)
    Tc = _roundup(chunk, 128)
    KC = Tc // 128

    key = (T, Tc)
    if key not in _cache:
        _cache[key] = _build(T, Tc)
    nc = _cache[key]

    # partition-major weight layouts (8KB+ DMA packets)
    wdr = _pm(Wd.astype(np.float16), HT)
    # Wu: [1024, 4096] -> [128, (half, dc, 2048)]
    wur = np.ascontiguousarray(
        Wu.astype(np.float16).reshape(DT, 128, 2, H // 2)
        .transpose(1, 2, 0, 3).reshape(128, 2 * DT * (H // 2)))
    bd_r = bd.reshape(1, D)
    bc_r = bc.reshape(1, D)
    bu_r = bu.astype(np.float16).reshape(1, H)
    emb16 = emb.astype(np.float16)
    wc16 = Wc.astype(np.float16)

    in_maps = []
    for c, (a, b) in enumerate(pairs):
        sa, sb = int(seq[a]), int(seq[b])
        xp = np.zeros((T, H), np.float16)
        xp[:sa] = x[a, :sa]
        xp[sa:sa + sb] = x[b, :sb]
        t = np.arange(T, dtype=np.int64)[:, None]
        mx = np.zeros((T, M), np.float16)
        mx[:, :P] = ((t >= lo[a][None, :]) & (t < hi[a][None, :]))
        mx[:, P:] = ((t - sa >= lo[b][None, :]) & (t - sa < hi[b][None, :])
                     & (t >= sa))
        # this core's flat chunk of embedding rows + one-hot sample mask
        rl = rows[chunk * c:chunk * (c + 1)]
        ll = labels[chunk * c:chunk * (c + 1)]
        ep = np.zeros((Tc, H), np.float16)
        ep[:len(rl)] = emb16[rl]
        cm = np.zeros((Tc, B), np.float16)
        cm[np.arange(len(ll)), ll] = 1.0
        aug = np.zeros((B + 2, M), np.float32)
        aug[a, :P] = ind[a] / clen_c[a]
        aug[b, P:] = ind[b] / clen_c[b]
        aug[B, :P] = cnt[a]
        aug[B, P:] = cnt[b]
        aug[B + 1, :P] = ind[a]
        aug[B + 1, P:] = ind[b]
        sinv = np.concatenate([1.0 / Sj[a], 1.0 / Sj[b]]).reshape(M, 1)
        in_maps.append({
            "xh": xp, "mxr": _pm(mx, KT), "eh": ep, "cmr": _pm(cm, KC),
            "wcc": _pm(wc16[:, 128 * c:128 * (c + 1)], HT),
            "wdr": wdr, "wur": wur,
            "bdr": bd_r, "bcr": bc_r, "bur": bu_r,
            "aug": aug, "sinv": sinv.astype(np.float32),
        })

    res = run_bass_kernel_spmd(nc, in_maps, core_ids=list(range(NC)))
    _cache["last_result"] = res

    out = np.empty((B, P, H), np.float32)
    for c, (a, b) in enumerate(pairs):
        o = res.results[c]["out"]
        out[a] = o[:P]
        out[b] = o[P:]
    return out



# revision 7
# speedup vs baseline: 1.0609x; 1.0609x over previous
"""Trainium2 Bass kernel for nn_ContextPromptGenerator.

Math restructure (as baseline): pooled bins are masked segment sums over
tokens, so the 0/1-mask matmul runs FIRST on [T, 4096] packed rows, then
the 4096->1024 down-projection runs on pooled rows only.

v2 changes over the 201us baseline:
- TP-pair: cores are paired {2g, 2g+1}; a pair shares its 4 samples
  (128 bins).  Each member computes ALL 128 pair bins but only HALF of
  the D dimension (Wd[:, 512m:512m+512], 4MB) and HALF of the H output
  (Wu[:, 2048m:+2048], 4MB) -- weight DMA drops 16MB -> 8MB/core.  The
  bin sums are exchanged via 2-rank AllGathers (2 x 256KB xsT chunks,
  1 x 128KB siluT) on replica groups [[0,1],[2,3],[4,5],[6,7]].
- ctx path: eh/cmask/Wc-half go fp8e4 (emb x64, Wc x16; compensated in
  the fp32 aug coefficients: /1024).  Error verified unchanged (3.7e-4).
  Phase B computes ctx_d[16, 512-half] locally from the AllReduced ctx
  sums with the fp8 Wc half -- the old cdg AllGather is gone.
- DMA: one sync-queue FIFO in priority order (masks/eh -> x -> Wd ->
  Wc -> Wu), 8-17MB resident tiles DMAd in 0.5-4MB pieces, so the x
  stream finishes ~60us in and the weight stream gates the D/U tail.
- X phase is split in 2 h-chunks so the first xsT AllGather overlaps
  the second half of the x stream; phase B is emitted after D-chunk 0
  to fill the AG/weight-stall gaps.

Sharding: data-parallel over samples for the bin sums (2 per core,
paired to minimize roundup128(max seq pair)); TP-pair for D/U; ctx
embedding rows flat-packed over all 8 cores + AllReduce.
"""

import numpy as np
import ml_dtypes
from contextlib import ExitStack

import concourse.bass as bass
import concourse.mybir as mybir
import concourse.tile as tile
from concourse import bacc
from concourse.masks import make_identity
from concourse.bass_utils import run_bass_kernel_spmd

F32 = mybir.dt.float32
F16 = mybir.dt.float16
F8 = mybir.dt.float8e4
F8NP = ml_dtypes.float8_e4m3

B, S, C, H, D, V, P = 16, 2048, 512, 4096, 1024, 32000, 32
NC = 8          # cores
SPC = 2         # samples per core
M = SPC * P     # 64 own bins per core
MB2 = 2 * M     # 128 bins per core-pair
HT = H // 128   # 32 h-tiles
DT = D // 128   # 8 d-tiles
DH = D // 2     # 512: d half (per pair member)
HH = H // 2     # 2048: h half (per pair member)
RG8 = [[0, 1, 2, 3, 4, 5, 6, 7]]
RG2 = [[0, 1], [2, 3], [4, 5], [6, 7]]
EH_SCALE = 64.0      # emb fp8 pre-scale
WC_SCALE = 16.0      # Wc fp8 pre-scale
CTX_SCALE = EH_SCALE * WC_SCALE

_cache = {}


def _build(T, Tc):
    """Per-core SPMD Bass program.  T = packed hidden rows per core
    (multiple of 128), Tc = packed context-emb rows per core."""
    nc = bacc.Bacc(None, target_bir_lowering=False, num_devices=NC)

    KT = T // 128
    KC = Tc // 128

    # ---- dram I/O ----
    xh_d = nc.dram_tensor("xh", [128, 2 * KT * HH], F16, kind="ExternalInput")
    mx_d = nc.dram_tensor("mxr", [128, KT * M], F16, kind="ExternalInput")
    eh_d = nc.dram_tensor("eh8", [128, KC * H], F8, kind="ExternalInput")
    cm_d = nc.dram_tensor("cm8", [128, KC * B], F8, kind="ExternalInput")
    wch_d = nc.dram_tensor("wch8", [128, HT * DH], F8, kind="ExternalInput")
    wdh_d = nc.dram_tensor("wdh", [128, HT * DH], F16, kind="ExternalInput")
    wuh_d = nc.dram_tensor("wuh", [128, DT * HH], F16, kind="ExternalInput")
    bd_d = nc.dram_tensor("bdh", [1, DH], F32, kind="ExternalInput")
    bc_d = nc.dram_tensor("bch", [1, DH], F32, kind="ExternalInput")
    bu_d = nc.dram_tensor("buh", [1, HH], F16, kind="ExternalInput")
    aug_d = nc.dram_tensor("aug", [B + 2, MB2], F32, kind="ExternalInput")
    sinv_d = nc.dram_tensor("sinv", [MB2, 1], F32, kind="ExternalInput")
    out_d = nc.dram_tensor("out", [MB2, HH], F32, kind="ExternalOutput")
    # collective bounce buffers (internal DRAM)
    csp_d = nc.dram_tensor("csp", [128, HT * B], F32)
    csr_d = nc.dram_tensor("csr", [128, HT * B], F32)
    xsp_d = [nc.dram_tensor(f"xsp{g}", [128, 16 * M], F16) for g in range(2)]
    xsg_d = [nc.dram_tensor(f"xsg{g}", [256, 16 * M], F16) for g in range(2)]
    stp_d = nc.dram_tensor("stp", [128, 4 * MB2], F16)
    stg_d = nc.dram_tensor("stg", [256, 4 * MB2], F16)

    with tile.TileContext(nc) as tc, ExitStack() as ctx:
        const = ctx.enter_context(tc.tile_pool(name="const", bufs=1))
        keep = ctx.enter_context(tc.tile_pool(name="keep", bufs=1))
        xpool = ctx.enter_context(tc.tile_pool(name="xpool", bufs=3))
        opool = ctx.enter_context(tc.tile_pool(name="opool", bufs=1))

        ident16 = const.tile([128, 128], F16)
        idtmp = const.tile([128, 128], F32)
        make_identity(nc, idtmp)
        nc.vector.tensor_copy(ident16, idtmp)
        ones1 = const.tile([1, 128], F16)
        nc.vector.memset(ones1, 1.0)

        # ---- sync-queue input DMAs, emitted in priority order ----
        cm8_sb = keep.tile([128, KC * B], F8)
        nc.sync.dma_start(out=cm8_sb, in_=cm_d[:, :])
        eh_sb = keep.tile([128, KC * H], F8)
        ehmid = (KC // 2) * H
        nc.sync.dma_start(out=eh_sb[:, 0:ehmid], in_=eh_d[:, 0:ehmid])
        nc.sync.dma_start(out=eh_sb[:, ehmid:], in_=eh_d[:, ehmid:])
        mxr_sb = keep.tile([128, KT * M], F16)
        nc.sync.dma_start(out=mxr_sb, in_=mx_d[:, :])
        aug_sb = keep.tile([B + 2, MB2], F32)
        nc.sync.dma_start(out=aug_sb, in_=aug_d[:, :])
        sinv_sb = keep.tile([MB2, 1], F32)
        nc.sync.dma_start(out=sinv_sb, in_=sinv_d[:, :])
        augr_sb = keep.tile([B + 2, DH], F32)
        nc.sync.dma_start(out=augr_sb[B:B + 1, :], in_=bd_d[:, :])
        nc.sync.dma_start(out=augr_sb[B + 1:B + 2, :], in_=bc_d[:, :])
        # weight tiles (DMAs emitted AFTER phase X so the sync-queue FIFO
        # streams x first, weights behind it)
        wdh_sb = keep.tile([128, HT * DH], F16)
        wch_sb = keep.tile([128, HT * DH], F8)
        wuh_sb = keep.tile([128, DT * HH], F16)
        buh_sb = keep.tile([1, HH], F16)

        cs32 = keep.tile([128, HT * B], F32)
        cs8 = keep.tile([128, HT * B], F8)
        xs_c = keep.tile([128, 2 * 16 * M], F16)
        xsA = [keep.tile([128, 16 * M], F16, name=f"xsA{g}") for g in range(2)]
        xsB = [keep.tile([128, 16 * M], F16, name=f"xsB{g}") for g in range(2)]
        silu_sb = keep.tile([MB2, DH], F16)
        st_own = keep.tile([128, 4 * MB2], F16)
        stA = keep.tile([128, 4 * MB2], F16)
        stB = keep.tile([128, 4 * MB2], F16)

        state = {"mm": 0}

        with tc.tile_pool(name="psA", bufs=1, space="PSUM") as psA, \
                tc.tile_pool(name="psX", bufs=1, space="PSUM") as psX, \
                tc.tile_pool(name="psB", bufs=1, space="PSUM") as psB:
            # ---- phase A: partial ctx_sumT[h, s] over this core's rows ----
            ps_ctx = psA.tile([128, HT * B], F32)
            for kc in range(KC):
                for hc in range(HT):
                    nc.tensor.matmul(
                        ps_ctx[:, B * hc:B * (hc + 1)],
                        eh_sb[:, kc * H + 128 * hc:kc * H + 128 * (hc + 1)],
                        cm8_sb[:, B * kc:B * (kc + 1)],
                        start=(kc == 0 and hc == 0),
                        stop=(kc == KC - 1),
                    )
            nc.vector.tensor_copy(cs32, ps_ctx)
            # AllReduce partial ctx sums (gpsimd queue)
            nc.gpsimd.dma_start(out=csp_d[:, :], in_=cs32)
            nc.gpsimd.collective_compute(
                "AllReduce", mybir.AluOpType.add, replica_groups=RG8,
                ins=[csp_d[:, :].opt()], outs=[csr_d[:, :].opt()])
            nc.gpsimd.dma_start(out=cs8, in_=csr_d[:, :])  # cast f32->f8

            # ---- phase B MMs (emitted later, interleaved): ctx_d half ----
            ps_cd = psB.tile([B, DH], F32)

            def emit_b(upto):
                while state["mm"] < min(HT, upto):
                    hc = state["mm"]
                    nc.tensor.matmul(
                        ps_cd,
                        cs8[:, B * hc:B * (hc + 1)],
                        wch_sb[:, DH * hc:DH * (hc + 1)],
                        start=(hc == 0),
                        stop=(hc == HT - 1),
                    )
                    state["mm"] = hc + 1

            # ---- phase X: xsumT[h, j] in 2 h-chunks of 16 h-tiles ----
            # x streams through xpool, 2 k-tiles (1MB) per buffer
            ps_xs = psX.tile([128, HT * M], F32)  # 4 banks
            KB = (KT + 1) // 2
            for g in range(2):
                for kb in range(KB):
                    nk = min(2, KT - 2 * kb)
                    xt = xpool.tile([128, 2 * HH], F16, tag="xkb")
                    nc.sync.dma_start(
                        out=xt[:, 0:nk * HH],
                        in_=xh_d[:, g * KT * HH + 2 * kb * HH:
                                 g * KT * HH + (2 * kb + nk) * HH])
                    for kk in range(nk):
                        k = 2 * kb + kk
                        for hcl in range(16):
                            nc.tensor.matmul(
                                ps_xs[:, (16 * g + hcl) * M:
                                      (16 * g + hcl + 1) * M],
                                xt[:, kk * HH + 128 * hcl:
                                   kk * HH + 128 * (hcl + 1)],
                                mxr_sb[:, M * k:M * (k + 1)],
                                start=(k == 0 and hcl % 8 == 0),
                                stop=(k == KT - 1),
                            )
                cg0, cg1 = g * 16 * M, (g + 1) * 16 * M
                nc.vector.tensor_copy(xs_c[:, cg0:cg1], ps_xs[:, cg0:cg1])
                nc.gpsimd.dma_start(out=xsp_d[g][:, :], in_=xs_c[:, cg0:cg1])
                nc.gpsimd.collective_compute(
                    "AllGather", mybir.AluOpType.bypass, replica_groups=RG2,
                    ins=[xsp_d[g][:, :].opt()], outs=[xsg_d[g][:, :].opt()])
                nc.gpsimd.dma_start(out=xsA[g], in_=xsg_d[g][0:128, :])
                nc.gpsimd.dma_start(out=xsB[g], in_=xsg_d[g][128:256, :])

            # weight DMAs: behind x on the sync queue, ahead of D/B/U use
            for q in range(4):
                c0, c1 = q * 8 * DH, (q + 1) * 8 * DH
                nc.sync.dma_start(out=wdh_sb[:, c0:c1], in_=wdh_d[:, c0:c1])
            nc.sync.dma_start(out=wch_sb, in_=wch_d[:, :])
            for q in range(4):
                c0, c1 = q * 2 * HH, (q + 1) * 2 * HH
                nc.sync.dma_start(out=wuh_sb[:, c0:c1], in_=wuh_d[:, c0:c1])
            nc.sync.dma_start(out=buh_sb, in_=bu_d[:, :])

            # ---- phase D chunk 0 (pair bins x d-half) ----
            ps_d = psB.tile([128, 2 * DH], F32)  # 2 banks, diag blocks
            for hc in range(16):
                nc.tensor.matmul(
                    ps_d[0:64, 0:DH],
                    xsA[0][:, M * hc:M * (hc + 1)],
                    wdh_sb[:, DH * hc:DH * (hc + 1)],
                    start=(hc == 0), stop=False)
                nc.tensor.matmul(
                    ps_d[64:128, DH:2 * DH],
                    xsB[0][:, M * hc:M * (hc + 1)],
                    wdh_sb[:, DH * hc:DH * (hc + 1)],
                    start=(hc == 0), stop=False)
            # phase B now (wch just landed; fills the AG1/wd stall)
            emit_b(HT)
            nc.vector.tensor_copy(augr_sb[0:B, :], ps_cd)
            # ---- phase D chunk 1 + aug ----
            for hcl in range(16):
                hc = 16 + hcl
                nc.tensor.matmul(
                    ps_d[0:64, 0:DH],
                    xsA[1][:, M * hcl:M * (hcl + 1)],
                    wdh_sb[:, DH * hc:DH * (hc + 1)],
                    start=False, stop=False)
                nc.tensor.matmul(
                    ps_d[64:128, DH:2 * DH],
                    xsB[1][:, M * hcl:M * (hcl + 1)],
                    wdh_sb[:, DH * hc:DH * (hc + 1)],
                    start=False, stop=False)
            nc.tensor.matmul(ps_d[0:64, 0:DH], aug_sb[:, 0:64], augr_sb,
                             start=False, stop=True)
            nc.tensor.matmul(ps_d[64:128, DH:2 * DH], aug_sb[:, 64:128],
                             augr_sb, start=False, stop=True)
            # scale by 1/S and silu
            nc.scalar.activation(
                silu_sb[0:64, :], ps_d[0:64, 0:DH],
                mybir.ActivationFunctionType.Silu, scale=sinv_sb[0:64, :])
            nc.scalar.activation(
                silu_sb[64:128, :], ps_d[64:128, DH:2 * DH],
                mybir.ActivationFunctionType.Silu, scale=sinv_sb[64:128, :])

        # ---- phase E: siluT [d-half, 128 bins] ----
        with tc.tile_pool(name="psE", bufs=2, space="PSUM") as psE:
            for dc in range(4):
                pst = psE.tile([128, 128], F16, tag="silutr")
                nc.tensor.transpose(
                    pst, silu_sb[:, 128 * dc:128 * (dc + 1)], ident16)
                nc.vector.tensor_copy(
                    st_own[:, 128 * dc:128 * (dc + 1)], pst)
        nc.gpsimd.dma_start(out=stp_d[:, :], in_=st_own)
        nc.gpsimd.collective_compute(
            "AllGather", mybir.AluOpType.bypass, replica_groups=RG2,
            ins=[stp_d[:, :].opt()], outs=[stg_d[:, :].opt()])
        nc.gpsimd.dma_start(out=stA, in_=stg_d[0:128, :])
        nc.gpsimd.dma_start(out=stB, in_=stg_d[128:256, :])

        # ---- phase U: out[pair bins, h-half] ----
        with tc.tile_pool(name="psU", bufs=1, space="PSUM") as psU:
            ps_u = psU.tile([MB2, HH], F32)  # 4 banks
            for dk in range(DT):
                sl = stA if dk < 4 else stB
                lhsT = sl[:, 128 * (dk % 4):128 * (dk % 4 + 1)]
                for nb in range(4):
                    nc.tensor.matmul(
                        ps_u[:, 512 * nb:512 * (nb + 1)],
                        lhsT,
                        wuh_sb[:, HH * dk + 512 * nb:HH * dk + 512 * (nb + 1)],
                        start=(dk == 0), stop=False)
            for nb in range(4):
                nc.tensor.matmul(
                    ps_u[:, 512 * nb:512 * (nb + 1)],
                    ones1,
                    buh_sb[:, 512 * nb:512 * (nb + 1)],
                    start=False, stop=True)
            ot = opool.tile([MB2, HH], F32)
            for nb in range(4):
                nc.vector.tensor_copy(
                    ot[:, 512 * nb:512 * (nb + 1)],
                    ps_u[:, 512 * nb:512 * (nb + 1)])
            nc.sync.dma_start(out=out_d[:, :], in_=ot)

    nc.finalize()
    return nc


def _roundup(v, m):
    return max(m, ((int(v) + m - 1) // m) * m)


def _pm(a, kt):
    """Reorder [kt*128, cols] row-major -> partition-major [128, kt*cols]."""
    n, cols = a.shape
    assert n == kt * 128
    return np.ascontiguousarray(
        a.reshape(kt, 128, cols).transpose(1, 0, 2).reshape(128, kt * cols))


def _f8(a):
    return np.ascontiguousarray(np.asarray(a, dtype=np.float32)).astype(F8NP)


def _pair_samples(seq):
    """Pair the 16 samples 2-per-core minimizing roundup128(max pair seq).
    Greedy sort-and-reflect, then 2-opt passes."""
    order = np.argsort(-seq, kind="stable")
    pairs = [[int(order[i]), int(order[2 * NC - 1 - i])] for i in range(NC)]

    def cost(ps):
        return (_roundup(max(seq[a] + seq[b] for a, b in ps), 128),
                max(seq[a] + seq[b] for a, b in ps))

    best = cost(pairs)
    improved = True
    while improved:
        improved = False
        for i in range(NC):
            for j in range(i + 1, NC):
                for swap in ((1, 1), (1, 0), (0, 1)):
                    cand = [list(p) for p in pairs]
                    cand[i][swap[0]], cand[j][swap[1]] = \
                        cand[j][swap[1]], cand[i][swap[0]]
                    c = cost(cand)
                    if c < best:
                        best, pairs, improved = c, cand, True
    return [(a, b) for a, b in pairs]


def kernel(**inputs):
    ids = np.asarray(inputs["context_ids"]).astype(np.int64)
    x = np.asarray(inputs["hidden_states"], dtype=np.float32)
    seq = np.asarray(inputs["seq_lengths"]).astype(np.int64)
    clen = np.asarray(inputs["context_lengths"]).astype(np.int64)
    emb = np.asarray(inputs["embed_table"], dtype=np.float32)
    Wc = np.ascontiguousarray(inputs["Wc"], dtype=np.float32)
    bc = np.asarray(inputs["bc"], dtype=np.float32)
    Wd = np.ascontiguousarray(inputs["Wd"], dtype=np.float32)
    bd = np.asarray(inputs["bd"], dtype=np.float32)
    Wu = np.ascontiguousarray(inputs["Wu"], dtype=np.float32)
    bu = np.asarray(inputs["bu"], dtype=np.float32)

    assert x.shape == (B, S, H) and ids.shape == (B, C)

    # per-sample bin geometry
    L = seq + 1
    jj = np.arange(P, dtype=np.int64)
    start = (jj[None, :] * L[:, None]) // P            # [B,P]
    end = ((jj[None, :] + 1) * L[:, None] + P - 1) // P
    Sj = (end - start).astype(np.float32)
    lo = np.maximum(start - 1, 0)
    hi = end - 1
    cnt = (hi - lo).astype(np.float32)
    ind = (start == 0).astype(np.float32)

    pairs = _pair_samples(seq)
    T = _roundup(max(seq[a] + seq[b] for a, b in pairs), 128)
    KT = T // 128

    # flat-pack ALL samples' context-embedding rows evenly across cores
    clen_c = np.maximum(clen, 1)
    labels = np.concatenate([np.full(int(clen_c[s]), s, np.int64)
                             for s in range(B)])
    rows = np.concatenate([ids[s, :int(clen_c[s])] for s in range(B)])
    tot = len(labels)
    chunk = -(-tot // NC)
    Tc = _roundup(chunk, 128)
    KC = Tc // 128

    key = (T, Tc)
    if key not in _cache:
        _cache[key] = _build(T, Tc)
    nc = _cache[key]

    # host-side weight layouts
    emb8 = _f8(emb * EH_SCALE)
    wd16 = Wd.astype(np.float16)
    wu16 = Wu.astype(np.float16)
    wdh = [_pm(np.ascontiguousarray(wd16[:, DH * m:DH * (m + 1)]), HT)
           for m in range(2)]
    wch = [_pm(_f8(Wc[:, DH * m:DH * (m + 1)] * WC_SCALE), HT)
           for m in range(2)]
    wuh = [np.ascontiguousarray(
        wu16[:, HH * m:HH * (m + 1)].reshape(DT, 128, HH)
        .transpose(1, 0, 2).reshape(128, DT * HH)) for m in range(2)]
    bdh = [bd[DH * m:DH * (m + 1)].reshape(1, DH) for m in range(2)]
    bch = [bc[DH * m:DH * (m + 1)].reshape(1, DH) for m in range(2)]
    buh = [bu[HH * m:HH * (m + 1)].astype(np.float16).reshape(1, HH)
           for m in range(2)]

    in_maps = []
    for c, (a, b) in enumerate(pairs):
        g, m = c // 2, c % 2
        sa, sb = int(seq[a]), int(seq[b])
        xp = np.zeros((T, H), np.float16)
        xp[:sa] = x[a, :sa]
        xp[sa:sa + sb] = x[b, :sb]
        # chunk-major reorder: [128, g(2) x KT x 2048]
        xr = np.ascontiguousarray(
            xp.reshape(KT, 128, 2, HH).transpose(1, 2, 0, 3)
            .reshape(128, 2 * KT * HH))
        t = np.arange(T, dtype=np.int64)[:, None]
        mx = np.zeros((T, M), np.float16)
        mx[:, :P] = ((t >= lo[a][None, :]) & (t < hi[a][None, :]))
        mx[:, P:] = ((t - sa >= lo[b][None, :]) & (t - sa < hi[b][None, :])
                     & (t >= sa))
        # this core's flat chunk of embedding rows + one-hot sample mask
        rl = rows[chunk * c:chunk * (c + 1)]
        ll = labels[chunk * c:chunk * (c + 1)]
        ep = np.zeros((Tc, H), F8NP)
        ep[:len(rl)] = emb8[rl]
        cm = np.zeros((Tc, B), np.float32)
        cm[np.arange(len(ll)), ll] = 1.0
        # group bin order: [pair0 sample a bins, pair0 b, pair1 a, pair1 b]
        gs = [pairs[2 * g][0], pairs[2 * g][1],
              pairs[2 * g + 1][0], pairs[2 * g + 1][1]]
        aug = np.zeros((B + 2, MB2), np.float32)
        sinv = np.zeros((MB2, 1), np.float32)
        for i, s in enumerate(gs):
            sl = slice(P * i, P * (i + 1))
            aug[s, sl] = ind[s] / (clen_c[s] * CTX_SCALE)
            aug[B, sl] = cnt[s]
            aug[B + 1, sl] = ind[s]
            sinv[sl, 0] = 1.0 / Sj[s]
        in_maps.append({
            "xh": xr, "mxr": _pm(mx, KT),
            "eh8": _pm(ep, KC), "cm8": _pm(cm.astype(F8NP), KC),
            "wch8": wch[m], "wdh": wdh[m], "wuh": wuh[m],
            "bdh": bdh[m], "bch": bch[m], "buh": buh[m],
            "aug": aug, "sinv": sinv,
        })

    res = run_bass_kernel_spmd(nc, in_maps, core_ids=list(range(NC)))
    _cache["last_result"] = res

    out = np.empty((B, P, H), np.float32)
    for c in range(NC):
        g, m = c // 2, c % 2
        o = res.results[c]["out"]          # [128 pair bins, 2048 h-half]
        gs = [pairs[2 * g][0], pairs[2 * g][1],
              pairs[2 * g + 1][0], pairs[2 * g + 1][1]]
        for i, s in enumerate(gs):
            out[s, :, HH * m:HH * (m + 1)] = o[P * i:P * (i + 1), :]
    return out


# revision 12
# speedup vs baseline: 1.0849x; 1.0227x over previous
"""Trainium2 Bass kernel for nn_ContextPromptGenerator.

Math restructure (as baseline): pooled bins are masked segment sums over
tokens, so the 0/1-mask matmul runs FIRST on [T, 4096] packed rows, then
the 4096->1024 down-projection runs on pooled rows only.

v3 over the 201us baseline:
- TP-pair: cores are paired {2g, 2g+1}; a pair shares its 4 samples
  (128 bins).  Each member computes ALL 128 pair bins but only HALF of
  the D dimension (Wd[:, 512m:+512], 4MB) and HALF of the H output
  (Wu[:, 2048m:+2048], 4MB) -- weight DMA drops 16MB -> 8MB/core.  Bin
  sums are exchanged via 2-rank AllGathers (2 x 256KB xsT chunks,
  1 x 128KB siluT) on replica groups [[0,1],[2,3],[4,5],[6,7]].
- ctx path is pair-local: each core sums ONLY its own 2 samples'
  context embeddings (no cross-core reduction exists, so the 42us
  8-rank AllReduce of v2 is gone); a 16KB pair-AllGather supplies the
  partner's sums; ctx_d[4, 512-half] comes from the fp8 Wc half.
- eh/cmask/Wc-half are fp8e4 (emb x64, Wc x16; compensated in the fp32
  aug coefficients /1024).  Verified: error unchanged (4.1e-4).
- Phase D runs full-array: the two xs slabs are interleaved on-chip
  (DVE strided copy) into [128, hc x 128bins] so each D matmul carries
  128 weight columns.
- DMA: one sync-queue FIFO in priority order (masks/eh -> x -> Wd ->
  Wc -> Wu); x streams through a 3-buffer pool in 1MB pieces; weights
  are 0.5-4MB pieces consumed slice-wise by D/U as they land.
- Output is written fp16 (host upcasts); halves the out DMA.

Sharding: data-parallel over samples for the bin sums (2 per core,
paired to minimize roundup128(max seq pair)); TP-pair for D/U and ctx.
"""

import numpy as np
import ml_dtypes
from contextlib import ExitStack

import concourse.bass as bass
import concourse.mybir as mybir
import concourse.tile as tile
from concourse import bacc
from concourse.masks import make_identity
from concourse.bass_utils import run_bass_kernel_spmd

F32 = mybir.dt.float32
F16 = mybir.dt.float16
F8 = mybir.dt.float8e4
F8NP = ml_dtypes.float8_e4m3

B, S, C, H, D, V, P = 16, 2048, 512, 4096, 1024, 32000, 32
NC = 8          # cores
SPC = 2         # samples per core
M = SPC * P     # 64 own bins per core
MB2 = 2 * M     # 128 bins per core-pair
HT = H // 128   # 32 h-tiles
DT = D // 128   # 8 d-tiles
DH = D // 2     # 512: d half (per pair member)
HH = H // 2     # 2048: h half (per pair member)
RG2 = [[0, 1], [2, 3], [4, 5], [6, 7]]
EH_SCALE = 64.0      # emb fp8 pre-scale
WC_SCALE = 16.0      # Wc fp8 pre-scale
CTX_SCALE = EH_SCALE * WC_SCALE

_cache = {}


def _build(T, Tc):
    """Per-core SPMD Bass program.  T = packed hidden rows per core,
    Tc = packed own-2-sample context rows per core (both mult. of 128)."""
    nc = bacc.Bacc(None, target_bir_lowering=False, num_devices=NC)

    KT = T // 128
    KC = Tc // 128

    # ---- dram I/O ----
    xh_d = nc.dram_tensor("xh", [128, 2 * KT * HH], F16, kind="ExternalInput")
    mx_d = nc.dram_tensor("mxr", [128, KT * M], F16, kind="ExternalInput")
    eh_d = nc.dram_tensor("eh8", [128, KC * H], F8, kind="ExternalInput")
    cm_d = nc.dram_tensor("cm8", [128, KC * SPC], F8, kind="ExternalInput")
    wch_d = nc.dram_tensor("wch8", [128, HT * DH], F8, kind="ExternalInput")
    wdh_d = nc.dram_tensor("wdh", [128, HT * DH], F16, kind="ExternalInput")
    wuh_d = nc.dram_tensor("wuh", [128, DT * HH], F16, kind="ExternalInput")
    bd_d = nc.dram_tensor("bdh", [1, DH], F32, kind="ExternalInput")
    bc_d = nc.dram_tensor("bch", [1, DH], F32, kind="ExternalInput")
    bu_d = nc.dram_tensor("buh", [1, HH], F16, kind="ExternalInput")
    aug_d = nc.dram_tensor("aug", [36, MB2], F32, kind="ExternalInput")
    sinv_d = nc.dram_tensor("sinv", [MB2, 1], F32, kind="ExternalInput")
    out_d = nc.dram_tensor("out", [MB2, HH], F16, kind="ExternalOutput")
    # collective bounce buffers (internal DRAM)
    csp_d = nc.dram_tensor("csp", [128, HT * SPC], F16)
    csg_d = nc.dram_tensor("csg", [256, HT * SPC], F16)
    xsp_d = [nc.dram_tensor(f"xsp{g}", [128, 16 * M], F16) for g in range(2)]
    xsg_d = [nc.dram_tensor(f"xsg{g}", [256, 16 * M], F16) for g in range(2)]
    stp_d = nc.dram_tensor("stp", [128, 4 * MB2], F16)
    stg_d = nc.dram_tensor("stg", [256, 4 * MB2], F16)

    with tile.TileContext(nc) as tc, ExitStack() as ctx:
        const = ctx.enter_context(tc.tile_pool(name="const", bufs=1))
        keep = ctx.enter_context(tc.tile_pool(name="keep", bufs=1))
        xpool = ctx.enter_context(tc.tile_pool(name="xpool", bufs=3))
        opool = ctx.enter_context(tc.tile_pool(name="opool", bufs=1))

        ident16 = const.tile([128, 128], F16)
        idtmp = const.tile([128, 128], F32)
        make_identity(nc, idtmp)
        nc.vector.tensor_copy(ident16, idtmp)
        ones1 = const.tile([1, 128], F16)
        nc.vector.memset(ones1, 1.0)

        # ---- sync-queue input DMAs, emitted in priority order ----
        cm8_sb = keep.tile([128, KC * SPC], F8)
        nc.sync.dma_start(out=cm8_sb, in_=cm_d[:, :])
        eh_sb = keep.tile([128, KC * H], F8)
        ehmid = (KC // 2) * H
        if ehmid > 0:
            nc.sync.dma_start(out=eh_sb[:, 0:ehmid], in_=eh_d[:, 0:ehmid])
        nc.sync.dma_start(out=eh_sb[:, ehmid:], in_=eh_d[:, ehmid:])
        mxr_sb = keep.tile([128, KT * M], F16)
        nc.sync.dma_start(out=mxr_sb, in_=mx_d[:, :])
        aug_sb = keep.tile([36, MB2], F32)
        nc.sync.dma_start(out=aug_sb, in_=aug_d[:, :])
        sinv_sb = keep.tile([MB2, 1], F32)
        nc.sync.dma_start(out=sinv_sb, in_=sinv_d[:, :])
        augr_sb = keep.tile([36, DH], F32)
        nc.vector.memset(augr_sb, 0.0)
        nc.sync.dma_start(out=augr_sb[34:35, :], in_=bd_d[:, :])
        nc.sync.dma_start(out=augr_sb[35:36, :], in_=bc_d[:, :])

        # weight tiles (DMAs emitted AFTER phase X: x streams first)
        wdh_sb = keep.tile([128, HT * DH], F16)
        wch_sb = keep.tile([128, HT * DH], F8)
        wuh_sb = keep.tile([128, DT * HH], F16)
        buh_sb = keep.tile([1, HH], F16)

        cs16 = keep.tile([128, HT * SPC], F16)
        csA8 = keep.tile([128, HT * SPC], F8)
        csB8 = keep.tile([128, HT * SPC], F8)
        xs_c = keep.tile([128, 2 * 16 * M], F16)
        xsA = [keep.tile([128, 16 * M], F16, name=f"xsA{g}") for g in range(2)]
        xsB = [keep.tile([128, 16 * M], F16, name=f"xsB{g}") for g in range(2)]
        xsAB = [keep.tile([128, 16 * 128], F16, name=f"xsAB{g}")
                for g in range(2)]
        silu_sb = keep.tile([MB2, DH], F16)
        st_own = keep.tile([128, 4 * MB2], F16)
        stA = keep.tile([128, 4 * MB2], F16)
        stB = keep.tile([128, 4 * MB2], F16)

        with tc.tile_pool(name="psA", bufs=1, space="PSUM") as psA, \
                tc.tile_pool(name="psX", bufs=1, space="PSUM") as psX, \
                tc.tile_pool(name="psB", bufs=1, space="PSUM") as psB, \
                tc.tile_pool(name="psD", bufs=1, space="PSUM") as psD:
            # ---- phase A: own-2-sample ctx_sumT[h, s], pair-AllGather ----
            ps_ctx = psA.tile([128, HT * SPC], F32)
            for kc in range(KC):
                for hc in range(HT):
                    nc.tensor.matmul(
                        ps_ctx[:, SPC * hc:SPC * (hc + 1)],
                        eh_sb[:, kc * H + 128 * hc:kc * H + 128 * (hc + 1)],
                        cm8_sb[:, SPC * kc:SPC * (kc + 1)],
                        start=(kc == 0 and hc == 0),
                        stop=(kc == KC - 1),
                    )
            nc.vector.tensor_copy(cs16, ps_ctx)
            nc.gpsimd.dma_start(out=csp_d[:, :], in_=cs16)
            nc.gpsimd.collective_compute(
                "AllGather", mybir.AluOpType.bypass, replica_groups=RG2,
                ins=[csp_d[:, :].opt()], outs=[csg_d[:, :].opt()])
            nc.gpsimd.dma_start(out=csA8, in_=csg_d[0:128, :])    # f16->f8
            nc.gpsimd.dma_start(out=csB8, in_=csg_d[128:256, :])  # f16->f8

            # ---- phase X: xsumT[h, j] in 2 h-chunks of 16 h-tiles ----
            ps_xs = psX.tile([128, HT * M], F32)  # 4 banks
            KB = (KT + 1) // 2
            for g in range(2):
                for kb in range(KB):
                    nk = min(2, KT - 2 * kb)
                    xt = xpool.tile([128, 2 * HH], F16, tag="xkb")
                    nc.sync.dma_start(
                        out=xt[:, 0:nk * HH],
                        in_=xh_d[:, g * KT * HH + 2 * kb * HH:
                                 g * KT * HH + (2 * kb + nk) * HH])
                    for kk in range(nk):
                        k = 2 * kb + kk
                        for hcl in range(16):
                            nc.tensor.matmul(
                                ps_xs[:, (16 * g + hcl) * M:
                                      (16 * g + hcl + 1) * M],
                                xt[:, kk * HH + 128 * hcl:
                                   kk * HH + 128 * (hcl + 1)],
                                mxr_sb[:, M * k:M * (k + 1)],
                                start=(k == 0 and hcl % 8 == 0),
                                stop=(k == KT - 1),
                            )
                cg0, cg1 = g * 16 * M, (g + 1) * 16 * M
                nc.vector.tensor_copy(xs_c[:, cg0:cg1], ps_xs[:, cg0:cg1])
                nc.gpsimd.dma_start(out=xsp_d[g][:, :], in_=xs_c[:, cg0:cg1])
                nc.gpsimd.collective_compute(
                    "AllGather", mybir.AluOpType.bypass, replica_groups=RG2,
                    ins=[xsp_d[g][:, :].opt()], outs=[xsg_d[g][:, :].opt()])
                nc.gpsimd.dma_start(out=xsA[g], in_=xsg_d[g][0:128, :])
                nc.gpsimd.dma_start(out=xsB[g], in_=xsg_d[g][128:256, :])
                # interleave the two slabs: [128, hc x (A 64 | B 64)]
                v = xsAB[g].rearrange("p (hc w) -> p hc w", w=128)
                nc.vector.tensor_copy(
                    v[:, :, 0:64],
                    xsA[g].rearrange("p (hc w) -> p hc w", w=64))
                nc.vector.tensor_copy(
                    v[:, :, 64:128],
                    xsB[g].rearrange("p (hc w) -> p hc w", w=64))

            # weight DMAs: behind x on the sync queue, ahead of D/B/U use
            for q in range(4):
                c0, c1 = q * 8 * DH, (q + 1) * 8 * DH
                nc.sync.dma_start(out=wdh_sb[:, c0:c1], in_=wdh_d[:, c0:c1])
            nc.sync.dma_start(out=wch_sb, in_=wch_d[:, :])
            for q in range(4):
                c0, c1 = q * 2 * HH, (q + 1) * 2 * HH
                nc.sync.dma_start(out=wuh_sb[:, c0:c1], in_=wuh_d[:, c0:c1])
            nc.sync.dma_start(out=buh_sb, in_=bu_d[:, :])

            # ---- phase D: pooled[128 pair bins, d-half], full-array ----
            ps_d = psD.tile([128, DH], F32)  # 1 bank
            for hc in range(HT):
                g, hcl = hc // 16, hc % 16
                nc.tensor.matmul(
                    ps_d,
                    xsAB[g][:, 128 * hcl:128 * (hcl + 1)],
                    wdh_sb[:, DH * hc:DH * (hc + 1)],
                    start=(hc == 0), stop=False)

            # ---- phase B: ctx_d[4 group samples, d-half] ----
            ps_cdA = psB.tile([2, DH], F32)  # 1 bank
            ps_cdB = psB.tile([2, DH], F32)  # 1 bank
            for hc in range(HT):
                nc.tensor.matmul(
                    ps_cdA,
                    csA8[:, SPC * hc:SPC * (hc + 1)],
                    wch_sb[:, DH * hc:DH * (hc + 1)],
                    start=(hc == 0), stop=(hc == HT - 1))
            for hc in range(HT):
                nc.tensor.matmul(
                    ps_cdB,
                    csB8[:, SPC * hc:SPC * (hc + 1)],
                    wch_sb[:, DH * hc:DH * (hc + 1)],
                    start=(hc == 0), stop=(hc == HT - 1))
            nc.vector.tensor_copy(augr_sb[0:2, :], ps_cdA)
            nc.vector.tensor_copy(augr_sb[32:34, :], ps_cdB)

            # aug term closes the D accumulation, then scaled silu
            nc.tensor.matmul(ps_d, aug_sb, augr_sb, start=False, stop=True)
            nc.scalar.activation(
                silu_sb, ps_d,
                mybir.ActivationFunctionType.Silu, scale=sinv_sb)

        # ---- phase E: siluT [d-half, 128 bins] ----
        with tc.tile_pool(name="psE", bufs=2, space="PSUM") as psE:
            for dc in range(4):
                pst = psE.tile([128, 128], F16, tag="silutr")
                nc.tensor.transpose(
                    pst, silu_sb[:, 128 * dc:128 * (dc + 1)], ident16)
                nc.vector.tensor_copy(
                    st_own[:, 128 * dc:128 * (dc + 1)], pst)
        nc.gpsimd.dma_start(out=stp_d[:, :], in_=st_own)
        nc.gpsimd.collective_compute(
            "AllGather", mybir.AluOpType.bypass, replica_groups=RG2,
            ins=[stp_d[:, :].opt()], outs=[stg_d[:, :].opt()])
        nc.gpsimd.dma_start(out=stA, in_=stg_d[0:128, :])
        nc.gpsimd.dma_start(out=stB, in_=stg_d[128:256, :])

        # ---- phase U: out[pair bins, h-half] ----
        with tc.tile_pool(name="psU", bufs=1, space="PSUM") as psU:
            ps_u = psU.tile([MB2, HH], F32)  # 4 banks
            for dk in range(DT):
                sl = stA if dk < 4 else stB
                lhsT = sl[:, 128 * (dk % 4):128 * (dk % 4 + 1)]
                for nb in range(4):
                    nc.tensor.matmul(
                        ps_u[:, 512 * nb:512 * (nb + 1)],
                        lhsT,
                        wuh_sb[:, HH * dk + 512 * nb:HH * dk + 512 * (nb + 1)],
                        start=(dk == 0), stop=False)
            for nb in range(4):
                nc.tensor.matmul(
                    ps_u[:, 512 * nb:512 * (nb + 1)],
                    ones1,
                    buh_sb[:, 512 * nb:512 * (nb + 1)],
                    start=False, stop=True)
            ot = opool.tile([MB2, HH], F16)
            for nb in range(4):
                nc.vector.tensor_copy(
                    ot[:, 512 * nb:512 * (nb + 1)],
                    ps_u[:, 512 * nb:512 * (nb + 1)])
            nc.sync.dma_start(out=out_d[:, :], in_=ot)

    nc.finalize()
    return nc


def _roundup(v, m):
    return max(m, ((int(v) + m - 1) // m) * m)


def _pm(a, kt):
    """Reorder [kt*128, cols] row-major -> partition-major [128, kt*cols]."""
    n, cols = a.shape
    assert n == kt * 128
    return np.ascontiguousarray(
        a.reshape(kt, 128, cols).transpose(1, 0, 2).reshape(128, kt * cols))


def _f8(a):
    return np.ascontiguousarray(np.asarray(a, dtype=np.float32)).astype(F8NP)


def _pair_samples(seq):
    """Pair the 16 samples 2-per-core minimizing roundup128(max pair seq).
    Greedy sort-and-reflect, then 2-opt passes."""
    order = np.argsort(-seq, kind="stable")
    pairs = [[int(order[i]), int(order[2 * NC - 1 - i])] for i in range(NC)]

    def cost(ps):
        return (_roundup(max(seq[a] + seq[b] for a, b in ps), 128),
                max(seq[a] + seq[b] for a, b in ps))

    best = cost(pairs)
    improved = True
    while improved:
        improved = False
        for i in range(NC):
            for j in range(i + 1, NC):
                for swap in ((1, 1), (1, 0), (0, 1)):
                    cand = [list(p) for p in pairs]
                    cand[i][swap[0]], cand[j][swap[1]] = \
                        cand[j][swap[1]], cand[i][swap[0]]
                    c = cost(cand)
                    if c < best:
                        best, pairs, improved = c, cand, True
    return [(a, b) for a, b in pairs]


def kernel(**inputs):
    ids = np.asarray(inputs["context_ids"]).astype(np.int64)
    x = np.asarray(inputs["hidden_states"], dtype=np.float32)
    seq = np.asarray(inputs["seq_lengths"]).astype(np.int64)
    clen = np.asarray(inputs["context_lengths"]).astype(np.int64)
    emb = np.asarray(inputs["embed_table"], dtype=np.float32)
    Wc = np.ascontiguousarray(inputs["Wc"], dtype=np.float32)
    bc = np.asarray(inputs["bc"], dtype=np.float32)
    Wd = np.ascontiguousarray(inputs["Wd"], dtype=np.float32)
    bd = np.asarray(inputs["bd"], dtype=np.float32)
    Wu = np.ascontiguousarray(inputs["Wu"], dtype=np.float32)
    bu = np.asarray(inputs["bu"], dtype=np.float32)

    assert x.shape == (B, S, H) and ids.shape == (B, C)

    # per-sample bin geometry
    L = seq + 1
    jj = np.arange(P, dtype=np.int64)
    start = (jj[None, :] * L[:, None]) // P            # [B,P]
    end = ((jj[None, :] + 1) * L[:, None] + P - 1) // P
    Sj = (end - start).astype(np.float32)
    lo = np.maximum(start - 1, 0)
    hi = end - 1
    cnt = (hi - lo).astype(np.float32)
    ind = (start == 0).astype(np.float32)

    pairs = _pair_samples(seq)
    T = _roundup(max(seq[a] + seq[b] for a, b in pairs), 128)
    KT = T // 128

    clen_c = np.maximum(clen, 1)
    Tc = _roundup(max(clen_c[a] + clen_c[b] for a, b in pairs), 128)
    KC = Tc // 128

    key = (T, Tc)
    if key not in _cache:
        _cache[key] = _build(T, Tc)
    nc = _cache[key]

    # host-side weight layouts
    emb8 = _f8(emb * EH_SCALE)
    wd16 = Wd.astype(np.float16)
    wu16 = Wu.astype(np.float16)
    wdh = [_pm(np.ascontiguousarray(wd16[:, DH * m:DH * (m + 1)]), HT)
           for m in range(2)]
    wch = [_pm(_f8(Wc[:, DH * m:DH * (m + 1)] * WC_SCALE), HT)
           for m in range(2)]
    wuh = [np.ascontiguousarray(
        wu16[:, HH * m:HH * (m + 1)].reshape(DT, 128, HH)
        .transpose(1, 0, 2).reshape(128, DT * HH)) for m in range(2)]
    bdh = [bd[DH * m:DH * (m + 1)].reshape(1, DH) for m in range(2)]
    bch = [bc[DH * m:DH * (m + 1)].reshape(1, DH) for m in range(2)]
    buh = [bu[HH * m:HH * (m + 1)].astype(np.float16).reshape(1, HH)
           for m in range(2)]

    in_maps = []
    for c, (a, b) in enumerate(pairs):
        g, m = c // 2, c % 2
        sa, sb = int(seq[a]), int(seq[b])
        xp = np.zeros((T, H), np.float16)
        xp[:sa] = x[a, :sa]
        xp[sa:sa + sb] = x[b, :sb]
        # chunk-major reorder: [128, g(2) x KT x 2048]
        xr = np.ascontiguousarray(
            xp.reshape(KT, 128, 2, HH).transpose(1, 2, 0, 3)
            .reshape(128, 2 * KT * HH))
        t = np.arange(T, dtype=np.int64)[:, None]
        mx = np.zeros((T, M), np.float16)
        mx[:, :P] = ((t >= lo[a][None, :]) & (t < hi[a][None, :]))
        mx[:, P:] = ((t - sa >= lo[b][None, :]) & (t - sa < hi[b][None, :])
                     & (t >= sa))
        # own 2 samples' context rows + 2-col one-hot
        ca, cb = int(clen_c[a]), int(clen_c[b])
        ep = np.zeros((Tc, H), F8NP)
        ep[:ca] = emb8[ids[a, :ca]]
        ep[ca:ca + cb] = emb8[ids[b, :cb]]
        cm = np.zeros((Tc, SPC), np.float32)
        cm[:ca, 0] = 1.0
        cm[ca:ca + cb, 1] = 1.0
        # group bin order: [pair0 sample a bins, pair0 b, pair1 a, pair1 b]
        gs = [pairs[2 * g][0], pairs[2 * g][1],
              pairs[2 * g + 1][0], pairs[2 * g + 1][1]]
        # augr rows: 0,1 = member0 ctx_d; 32,33 = member1 ctx_d (32-aligned
        # partition bases for the DVE copies); 34 = bd; 35 = bc
        aug = np.zeros((36, MB2), np.float32)
        sinv = np.zeros((MB2, 1), np.float32)
        for i, s in enumerate(gs):
            sl = slice(P * i, P * (i + 1))
            aug[i if i < 2 else 30 + i, sl] = ind[s] / (clen_c[s] * CTX_SCALE)
            aug[34, sl] = cnt[s]
            aug[35, sl] = ind[s]
            sinv[sl, 0] = 1.0 / Sj[s]
        in_maps.append({
            "xh": xr, "mxr": _pm(mx, KT),
            "eh8": _pm(ep, KC), "cm8": _pm(cm.astype(F8NP), KC),
            "wch8": wch[m], "wdh": wdh[m], "wuh": wuh[m],
            "bdh": bdh[m], "bch": bch[m], "buh": buh[m],
            "aug": aug, "sinv": sinv,
        })

    res = run_bass_kernel_spmd(nc, in_maps, core_ids=list(range(NC)))
    _cache["last_result"] = res

    out = np.empty((B, P, H), np.float32)
    for c in range(NC):
        g, m = c // 2, c % 2
        o = np.asarray(res.results[c]["out"], dtype=np.float32)
        gs = [pairs[2 * g][0], pairs[2 * g][1],
              pairs[2 * g + 1][0], pairs[2 * g + 1][1]]
        for i, s in enumerate(gs):
            out[s, :, HH * m:HH * (m + 1)] = o[P * i:P * (i + 1), :]
    return out
